# revision 1
# baseline (speedup 1.0000x reference)
"""Trainium2 Bass kernel for CostVolumePrompt (masked-softmax cost volume).

Computation per (b, h):
  vol[i, j] = dot(lfeat[b,:,h,i], rfeat[b,:,h,j]) / sqrt(C)      (W x W)
  prob      = softmax(vol, axis=j) * tril(W, W)                  (mask AFTER softmax)
  corresp_i = sum_j prob[i,j] * j
  conf_i    = max_j prob[i,j]
  disp_i    = max((i - corresp_i)/W, 0.1)       (corresp_i <= i, so abs == i-corresp)
  out       = [fx*baseline/lfar / disp, conf]

Device strategy (8 cores, data-parallel over H):
  Each core owns H/8 = 16 rows for all 4 batches -> 64 (b,h) pairs.
  Per pair, 4 row-tiles of (128 i x 512 j):
    PE  : vol tile via fp32r matmul (lhsT = lfeat chunk, rhs = rfeat row)
    ACT : e = exp(vol/sqrt(C)) with accum_out -> denominator (free)
    DVE : tensor_tensor_reduce(e * widx)      -> masked weighted sum s1
          tensor_mask_reduce(max over masked e) -> numerator of conf
  Tiny per-row finals are batched per b on (128, 64) tiles.
"""

import math
import numpy as np
from contextlib import ExitStack

import concourse.bass as bass
import concourse.bacc as bacc
import concourse.tile as tile
from concourse import mybir
from concourse._compat import with_exitstack
from concourse.bass_utils import run_bass_kernel_spmd
from concourse.dve_ops import TENSOR_TENSOR_REDUCE, TENSOR_MASK_REDUCE

B, V, C, H, W = 4, 2, 128, 128, 512
NCORES = 8
HLOC = H // NCORES          # 16 h-rows per core
HGRP = 8                    # h-rows per DMA group
NT = HLOC * 4               # finals columns per batch (h*4 + mi) = 64
SCALE = 1.0 / math.sqrt(C)  # 1/sqrt(C) / TEMPERATURE
MIN_DISP = 0.1

F32 = mybir.dt.float32
F32R = mybir.dt.float32r
F16 = mybir.dt.float16


@with_exitstack
def _body(ctx: ExitStack, tc: "tile.TileContext", io: dict):
    nc = tc.nc
    lfeat, rfeat = io["lfeat"], io["rfeat"]
    widx, maskend, ivec, scales = io["widx"], io["maskend"], io["ivec"], io["scales"]
    out_dc, out_cf = io["out_dc"], io["out_cf"]

    singles = ctx.enter_context(tc.tile_pool(name="singles", bufs=1))
    feats = ctx.enter_context(tc.tile_pool(name="feats", bufs=6))
    psum = ctx.enter_context(tc.tile_pool(name="psum", bufs=5, space="PSUM"))
    psacc = ctx.enter_context(tc.tile_pool(name="psacc", bufs=2, space="PSUM"))
    epool = ctx.enter_context(tc.tile_pool(name="epool", bufs=6))
    scr = ctx.enter_context(tc.tile_pool(name="scr", bufs=3))
    acc = ctx.enter_context(tc.tile_pool(name="acc", bufs=2))
    fin = ctx.enter_context(tc.tile_pool(name="fin", bufs=2))

    widx_sb = singles.tile([128, 4, W], F32)
    nc.sync.dma_start(out=widx_sb[:], in_=widx[:, :, :])
    maskend_sb = singles.tile([128, 4], F32)
    nc.sync.dma_start(out=maskend_sb[:], in_=maskend[:, :])
    ivec_sb = singles.tile([128, NT], F32)
    nc.sync.dma_start(out=ivec_sb[:], in_=ivec[:, :])
    sc_sb = singles.tile([128, B], F32)
    nc.gpsimd.dma_start(out=sc_sb[:], in_=scales[:, :].to_broadcast((128, B)))

    for b in range(B):
        s1c = acc.tile([128, NT], F32, tag="s1c")
        dnc = psacc.tile([128, NT], F32, tag="dnc")
        c1c = acc.tile([128, NT], F32, tag="c1c")
        for h in range(HLOC):
            lf = feats.tile([128, W], F16, tag="lf")
            rf = feats.tile([128, W], F16, tag="rf")
            if b == 0 and h < 2:
                # HWDGE fp32 load + ScalarE cast: skips the slow SWDGE
                # cast-DMA warmup on the critical first tiles.
                lf32 = feats.tile([128, W], F32, tag="lf32")
                rf32 = feats.tile([128, W], F32, tag="rf32")
                nc.sync.dma_start(out=lf32[:], in_=lfeat[b, :, h, :])
                nc.sync.dma_start(out=rf32[:], in_=rfeat[b, :, h, :])
                nc.scalar.copy(out=lf[:], in_=lf32[:])
                nc.scalar.copy(out=rf[:], in_=rf32[:])
            else:
                nc.gpsimd.dma_start(out=lf[:], in_=lfeat[b, :, h, :])
                nc.gpsimd.dma_start(out=rf[:], in_=rfeat[b, :, h, :])
            if True:
                for mi in range(4):
                    t = h * 4 + mi
                    ext = (mi + 1) * 128
                    vol = psum.tile([128, W], F32, tag="vol")
                    nc.tensor.matmul(
                        vol[:],
                        lf[:, mi * 128:(mi + 1) * 128],
                        rf[:, :],
                        start=True,
                        stop=True,
                    )
                    e = epool.tile([128, W], F32, tag="e")
                    nc.scalar.activation(
                        out=e[:],
                        in_=vol[:],
                        func=mybir.ActivationFunctionType.Exp,
                        scale=SCALE,
                        accum_out=dnc[:, t:t + 1],
                    )
                    so = scr.tile([128, W], F32, tag="so")
                    nc.vector._custom_dve(
                        TENSOR_TENSOR_REDUCE,
                        out=so[:, :ext],
                        in0=e[:, :ext],
                        in1=widx_sb[:, mi, :ext],
                        s0=0.0,      # accum seed
                        s1=1.0,      # scale
                        accum_out=s1c[:, t:t + 1],
                    )
                    mo = scr.tile([128, W], F32, tag="mo")
                    nc.vector._custom_dve(
                        TENSOR_MASK_REDUCE,
                        out=mo[:, :ext],
                        in0=e[:, :ext],
                        in1=maskend_sb[:, mi:mi + 1],   # mask_end (C3 spill)
                        s0=0.0,      # mask_start
                        s1=0.0,      # accum seed (e > 0 always)
                        imm2=1.0,    # scale
                        accum_out=c1c[:, t:t + 1],
                    )
        # ---- batched finals for this b ----
        r = fin.tile([128, NT], F32, tag="r")
        nc.vector.reciprocal_approx_fast(out=r[:], in_=dnc[:])
        cf = fin.tile([128, NT], F32, tag="cf")
        nc.gpsimd.tensor_mul(out=cf[:], in0=c1c[:], in1=r[:])
        cor = fin.tile([128, NT], F32, tag="cor")
        nc.gpsimd.tensor_mul(out=cor[:], in0=s1c[:], in1=r[:])
        dd = fin.tile([128, NT], F32, tag="dd")
        nc.gpsimd.tensor_sub(out=dd[:], in0=ivec_sb[:], in1=cor[:])
        dcl = fin.tile([128, NT], F32, tag="dcl")
        nc.gpsimd.tensor_scalar(
            out=dcl[:], in0=dd[:], scalar1=1.0 / W, scalar2=MIN_DISP,
            op0=mybir.AluOpType.mult, op1=mybir.AluOpType.max,
        )
        r2 = fin.tile([128, NT], F32, tag="r2")
        nc.vector.reciprocal_approx_fast(out=r2[:], in_=dcl[:])
        od = fin.tile([128, NT], F32, tag="od")
        nc.vector.tensor_scalar(
            out=od[:], in0=r2[:], scalar1=sc_sb[:, b:b + 1], scalar2=None,
            op0=mybir.AluOpType.mult,
        )
        nc.sync.dma_start(out=out_dc[b, :, :], in_=od[:])
        nc.sync.dma_start(out=out_cf[b, :, :], in_=cf[:])


_NC_CACHE = None


def _build_nc():
    global _NC_CACHE
    if _NC_CACHE is not None:
        return _NC_CACHE
    nc = bacc.Bacc("TRN2", target_bir_lowering=False, debug=False)
    io = {
        "lfeat": nc.dram_tensor("lfeat", (B, C, HLOC, W), F32, kind="ExternalInput"),
        "rfeat": nc.dram_tensor("rfeat", (B, C, HLOC, W), F32, kind="ExternalInput"),
        "widx": nc.dram_tensor("widx", (128, 4, W), F32, kind="ExternalInput"),
        "maskend": nc.dram_tensor("maskend", (128, 4), F32, kind="ExternalInput"),
        "ivec": nc.dram_tensor("ivec", (128, NT), F32, kind="ExternalInput"),
        "scales": nc.dram_tensor("scales", (1, B), F32, kind="ExternalInput"),
        "out_dc": nc.dram_tensor("out_dc", (B, 128, NT), F32, kind="ExternalOutput"),
        "out_cf": nc.dram_tensor("out_cf", (B, 128, NT), F32, kind="ExternalOutput"),
    }
    with tile.TileContext(nc) as tc:
        _body(tc, io)
    nc.compile()
    _NC_CACHE = nc
    return nc


def _host_constants():
    p = np.arange(128)[:, None, None]
    mi = np.arange(4)[None, :, None]
    j = np.arange(W)[None, None, :]
    widx = np.where(j <= mi * 128 + p, j, 0).astype(np.float32)       # (128,4,W)
    maskend = (np.arange(4)[None, :] * 128 + np.arange(128)[:, None] + 1).astype(
        np.float32)                                                    # (128,4)
    iv = (np.arange(4)[None, :] * 128 + np.arange(128)[:, None]).astype(np.float32)
    ivec = np.tile(iv, (1, HLOC))                                      # (128, 64)
    return widx, maskend, ivec


def kernel(feat, extri, intri, near, far, _run_kwargs=None, _core_ids=None):
    feat = np.asarray(feat, dtype=np.float32)
    extri = np.asarray(extri, dtype=np.float32)
    intri = np.asarray(intri, dtype=np.float32)
    far = np.asarray(far, dtype=np.float32)

    fx = intri[:, 0, 0, 0]                                             # (B,)
    baseline = np.linalg.norm(extri[:, 0, :3, 3] - extri[:, 1, :3, 3], axis=-1)
    lfar = far[:, 0]
    scales = (fx * baseline / lfar).astype(np.float32).reshape(1, B)

    widx, maskend, ivec = _host_constants()
    core_ids = list(range(NCORES)) if _core_ids is None else _core_ids

    in_maps = []
    for ci in range(len(core_ids)):
        hs = slice(ci * HLOC, (ci + 1) * HLOC)
        in_maps.append({
            "lfeat": np.ascontiguousarray(feat[:, 0, :, hs, :]),
            "rfeat": np.ascontiguousarray(feat[:, 1, :, hs, :]),
            "widx": widx, "maskend": maskend, "ivec": ivec, "scales": scales,
        })

    nc = _build_nc()
    res = run_bass_kernel_spmd(nc, in_maps, core_ids=core_ids,
                               **(_run_kwargs or {}))

    out = np.zeros((B, 1, 2, H, W), dtype=np.float32)
    for ci in range(len(core_ids)):
        h0 = ci * HLOC
        dc = res.results[ci]["out_dc"]          # (B, 128, 64), col = h*4+mi
        cf = res.results[ci]["out_cf"]
        dc = dc.reshape(B, 128, HLOC, 4).transpose(0, 2, 3, 1).reshape(B, HLOC, W)
        cf = cf.reshape(B, 128, HLOC, 4).transpose(0, 2, 3, 1).reshape(B, HLOC, W)
        out[:, 0, 0, h0:h0 + HLOC, :] = dc
        out[:, 0, 1, h0:h0 + HLOC, :] = cf
    if _run_kwargs:
        kernel.last_results = res
    return out



# revision 24
# speedup vs baseline: 1.0340x; 1.0340x over previous
"""Trainium2 Bass kernel for CostVolumePrompt (masked-softmax cost volume).

Computation per (b, h):
  vol[i, j] = dot(lfeat[b,:,h,i], rfeat[b,:,h,j]) / sqrt(C)      (W x W)
  prob      = softmax(vol, axis=j) * tril(W, W)                  (mask AFTER softmax)
  corresp_i = sum_j prob[i,j] * j
  conf_i    = max_j prob[i,j]
  disp_i    = max((i - corresp_i)/W, 0.1)
  out       = [fx*baseline/lfar / disp, conf]

Device strategy (8 cores, data-parallel over H): each core owns H/8 = 16
rows for all 4 batches -> 64 (b,h) pairs.  Per pair, 4 row-tiles of
(128 i x 512 j):
  PE  : vol tile via fp16 matmul (inputs shipped as fp16 from host)
  ACT : e = exp(vol*scale) -> fp16 SBUF, accum_out -> denominator (free)
  DVE : two custom ops, each with a hand-written 2x_1P uop program
        (2 fp16 elements/cycle -- the stock custom-DVE path only emits
        1x programs):
          COSTVOL_TTR2X: out = e*w, accum += sum   (w = j masked, fp16)
          COSTVOL_TMR2X: masked max of e via a counter-compare against
                         per-row bounds carried in s0/s1 (scaled by HUGE)
Final per-row math batched per b on (128, 64) tiles.
"""

import math
import numpy as np
from contextlib import ExitStack
from operator import add as _op_add

import concourse.bass as bass
import concourse.bacc as bacc
import concourse.tile as tile
from concourse import mybir
from concourse._compat import with_exitstack
from concourse.bass_utils import run_bass_kernel_spmd

import concourse.dve_ops as dve_ops
from concourse.dve_ops import DveOp
from concourse.dve_spec import Spec, Src0, Src1, C0, C1, C2, Zero, minn, maxx, lower, scan
from concourse.dve_uop import (
    UopConfig,
    UopDpConfig,
    DveOpSpec,
    AluOp,
    AluInp,
    InpSel,
    OutSel,
    OutPath,
    Trigger,
    DelayInp,
)

B, V, C, H, W = 4, 2, 128, 128, 512
NCORES = 8
HLOC = H // NCORES          # 16 h-rows per core
NT = HLOC * 4               # finals columns per batch (h*4 + mi) = 64
SCALE = 1.0 / math.sqrt(C)  # 1/sqrt(C) / TEMPERATURE
MIN_DISP = 0.1
HUGE = float(2.0 ** 100)    # mask sentinel scale; (end-k)*HUGE stays exact fp32
WSCALE = 64.0               # widx carries j/WSCALE so fp16 running sums fit

F32 = mybir.dt.float32
F16 = mybir.dt.float16

USE_2X = True               # set False to fall back to the 1x programs


# --------------------------------------------------------------------------- #
# Custom DVE ops with hand-authored 2x_1P uop programs.
#
# The 1x program is compiled from the Spec DSL by lower(); the 2x program is
# written by hand below (the framework's T1 "2x perf mode" is unimplemented,
# but dve_table_gen + the instruction encoder fully support uops_2x +
# perf_max).  The instruction scalars are chosen so that BOTH programs
# compute the same result (the engine silently falls back to 1x if the
# access pattern doesn't qualify):
#   TTR2X: masked weighted sum; the mask lives in the weight data.
#   TMR2X: masked max over k < end, end = endlo + endhi, with
#          s0 = endlo*HUGE, s1 = endhi*HUGE, imm2 = HUGE.
#          1x: m_k = min(x_k, (s0 - k*HUGE) + s1)         (k element idx)
#          2x: m_lo = min(x_2k, s0 - k*HUGE), m_hi = min(x_2k+1, s1 - k*HUGE)
#          (endlo = ceil(end/2), endhi = floor(end/2))
# --------------------------------------------------------------------------- #

_D = AluInp
_PD = DelayInp.PREV_DELAY
_PA = DelayInp.PREV_ALU_OUT


def _stage(u, k, op, a, b=None, *, out_a=False):
    blk = u.datapath_config[k]
    blk.op = op
    blk.alu_src0 = a
    blk.alu_src1 = b if b is not None else a
    blk.alu_out_enable = 1
    blk.alu_out_a_enable = 1 if out_a else 0
    return blk


def _lanes(u, k, pairs):
    blk = u.datapath_config[k]
    for ln, src in pairs:
        blk.delay[ln] = src
        blk.delay_enable[ln] = 1


def _ttr2x_uops():
    # Running weighted sum, readout = last out element (no DVE accumulator).
    # lanes: 0=Src0lo 1=Src1lo 2=Src0hi 3=Src1hi
    inps = [InpSel.ZERO, InpSel.SRC_0, InpSel.SRC_1, InpSel.SRC_0_HI,
            InpSel.SRC_1_HI, InpSel.ZERO, InpSel.ZERO]
    ens = [0, 1, 1, 1, 1, 0, 0]

    def base():
        u = UopConfig()
        for i in range(7):
            u.inp[i] = inps[i]
            u.inp_enable[i] = ens[i]
        return u

    seed = base()
    # init the running-sum flop (s3) to 0 via x^x; 1-cycle, non-consuming
    for k in range(8):
        _stage(seed, k, AluOp.BYPASS, _D.PREV_ALU_OUT)
    _stage(seed, 3, AluOp.BITWISE_XOR, _D.PREV_ALU_OUT, _D.PREV_ALU_OUT)
    seed.repeat_count = 1
    seed.trigger = (Trigger.COUNT, Trigger.NONE, Trigger.NONE)
    seed.next_uop = (1, 0, 0)

    st = base()
    _stage(st, 0, AluOp.MULTIPLY, _D.PREV_DELAY_0, _D.PREV_DELAY_1)
    _lanes(st, 0, [(2, _PD), (3, _PD)])
    _stage(st, 1, AluOp.MULTIPLY, _D.PREV_DELAY_2, _D.PREV_DELAY_3)
    _lanes(st, 1, [(0, _PA)])                     # park lo product
    _stage(st, 2, AluOp.ADD, _D.PREV_ALU_OUT, _D.PREV_DELAY_0)
    _stage(st, 3, AluOp.ADD, _D.CURR_ALU_OUT, _D.PREV_ALU_OUT)  # running sum
    for k in (4, 5, 6, 7):
        _stage(st, k, AluOp.BYPASS, _D.PREV_ALU_OUT)
    st.require_inp0 = 1
    st.require_inp1 = 1
    st.trigger = (Trigger.SRC_TENSOR_DONE, Trigger.NONE, Trigger.NONE)
    st.next_uop = (0, 0, 0)
    st.out[OutPath.WR0_LO] = OutSel.ALU_OUT
    st.out_enable[OutPath.WR0_LO] = 1
    st.out[OutPath.WR0_HI] = OutSel.ALU_OUT
    st.out_enable[OutPath.WR0_HI] = 1
    return [seed, st]


def _tmr2x_uops():
    # lanes: 0=lo 1=hi 2=C0 3=C1 4=C2(HUGE) 5=ZERO
    # The op is made formally two-source (in1 is consumed but unused): a
    # single-source op gets pm=OneSrc, which lets the engine escalate to the
    # 2-port perf modes whose stream semantics this program does not
    # implement.  pm=TwoSrc caps the engine at 2X_1PORT.
    inps = [InpSel.ZERO, InpSel.SRC_0, InpSel.SRC_0_HI, InpSel.CONST_0,
            InpSel.CONST_1, InpSel.CONST_2, InpSel.ZERO]
    ens = [0, 1, 1, 1, 1, 1, 1]

    def base():
        u = UopConfig()
        for i in range(7):
            u.inp[i] = inps[i]
            u.inp_enable[i] = ens[i]
        u.accum_enabled = 1
        return u

    seed = base()
    # counter flop (s0) <- 0 - HUGE; running-max flop (s6) <- 0 via x^x.
    seed.inp_enable[0] = 1          # ZERO -> stage0 PREV_ALU_OUT
    _stage(seed, 0, AluOp.SUBTRACT, _D.PREV_ALU_OUT, _D.PREV_DELAY_4)
    for k in (1, 2, 3, 4, 5, 7):
        _stage(seed, k, AluOp.BYPASS, _D.PREV_ALU_OUT)
    _stage(seed, 6, AluOp.BITWISE_XOR, _D.PREV_ALU_OUT, _D.PREV_ALU_OUT)
    seed.repeat_count = 1
    seed.trigger = (Trigger.COUNT, Trigger.NONE, Trigger.NONE)
    seed.next_uop = (1, 0, 0)

    st = base()
    st.inp[6] = InpSel.SRC_1      # consume src1 (unused) -> pm=TwoSrc
    st.inp_enable[6] = 1
    st.require_inp0 = 1
    st.require_inp1 = 1
    _stage(st, 0, AluOp.ADD, _D.CURR_ALU_OUT, _D.PREV_DELAY_4)   # t += HUGE
    _lanes(st, 0, [(0, _PD), (1, _PD), (2, _PD), (3, _PD)])
    _stage(st, 1, AluOp.SUBTRACT, _D.PREV_DELAY_2, _D.PREV_ALU_OUT)  # d_lo
    _lanes(st, 1, [(0, _PD), (1, _PD), (2, _PA), (3, _PD)])      # park t
    _stage(st, 2, AluOp.SUBTRACT, _D.PREV_DELAY_3, _D.PREV_DELAY_2)  # d_hi
    _lanes(st, 2, [(0, _PD), (1, _PD), (3, _PA)])                # park d_lo
    _stage(st, 3, AluOp.MIN, _D.PREV_DELAY_0, _D.PREV_DELAY_3)   # m_lo
    _lanes(st, 3, [(1, _PD), (2, _PA)])                          # park d_hi
    _stage(st, 4, AluOp.MIN, _D.PREV_DELAY_1, _D.PREV_DELAY_2)   # m_hi
    _lanes(st, 4, [(0, _PA)])                                    # park m_lo
    _stage(st, 5, AluOp.MAX, _D.PREV_ALU_OUT, _D.PREV_DELAY_0)   # mx
    _stage(st, 6, AluOp.MAX, _D.CURR_ALU_OUT, _D.PREV_ALU_OUT)   # running max
    _stage(st, 7, AluOp.BYPASS, _D.PREV_ALU_OUT)
    st.trigger = (Trigger.SRC_TENSOR_DONE, Trigger.NONE, Trigger.NONE)
    st.next_uop = (0, 0, 0)
    st.out[OutPath.WR0_LO] = OutSel.ALU_OUT
    st.out_enable[OutPath.WR0_LO] = 1
    st.out[OutPath.WR0_HI] = OutSel.ALU_OUT
    st.out_enable[OutPath.WR0_HI] = 1
    return [seed, st]


def _ttr_ref(in0, in1, s0, s1, imm2):
    P = in0.shape[0]
    b = (in0.astype(np.float32) * np.asarray(in1, np.float32)).reshape(P, -1)
    return np.cumsum(b, axis=-1).reshape(in0.shape).astype(np.float32)


def _tmr_ref(in0, in1, s0, s1, imm2):
    P = in0.shape[0]
    x = in0.astype(np.float32).reshape(P, -1)
    N = x.shape[1]
    k = np.arange(N, dtype=np.float32)[None, :]
    c0 = np.asarray(s0, np.float32).reshape(-1, 1)
    c1 = np.asarray(s1, np.float32).reshape(-1, 1)
    d = (c0 - k * np.float32(imm2)) + c1
    body = np.minimum(x, d)
    run = np.maximum(np.maximum.accumulate(body, axis=-1), 0.0)
    return run.reshape(in0.shape).astype(np.float32)


def _tmr1x_uops():
    # 1x fallback for COSTVOL_TMR2X (the DSL can't express the nested scan).
    # lanes: 0=SRC_0 1=C0 2=C1 3=C2(HUGE) 4=ZERO 5=SRC_1(steady)
    inps = [InpSel.ZERO, InpSel.SRC_0, InpSel.CONST_0, InpSel.CONST_1,
            InpSel.CONST_2, InpSel.ZERO, InpSel.ZERO]
    ens = [0, 1, 1, 1, 1, 1, 0]

    def base():
        u = UopConfig()
        for i in range(7):
            u.inp[i] = inps[i]
            u.inp_enable[i] = ens[i]
        return u

    seed = base()
    seed.inp_enable[0] = 1          # ZERO -> stage0 PREV_ALU_OUT
    _stage(seed, 0, AluOp.SUBTRACT, _D.PREV_ALU_OUT, _D.PREV_DELAY_3)  # -HUGE
    for k in (1, 2, 3, 5, 6, 7):
        _stage(seed, k, AluOp.BYPASS, _D.PREV_ALU_OUT)
    _stage(seed, 4, AluOp.BITWISE_XOR, _D.PREV_ALU_OUT, _D.PREV_ALU_OUT)
    seed.repeat_count = 1
    seed.trigger = (Trigger.COUNT, Trigger.NONE, Trigger.NONE)
    seed.next_uop = (1, 0, 0)

    st = base()
    st.inp[6] = InpSel.SRC_1
    st.inp_enable[6] = 1
    st.require_inp0 = 1
    st.require_inp1 = 1
    _stage(st, 0, AluOp.ADD, _D.CURR_ALU_OUT, _D.PREV_DELAY_3)   # t += HUGE
    _lanes(st, 0, [(0, _PD), (1, _PD), (2, _PD)])
    _stage(st, 1, AluOp.SUBTRACT, _D.PREV_DELAY_1, _D.PREV_ALU_OUT)  # C0 - t
    _lanes(st, 1, [(0, _PD), (2, _PD)])
    _stage(st, 2, AluOp.ADD, _D.PREV_ALU_OUT, _D.PREV_DELAY_2)   # + C1
    _lanes(st, 2, [(0, _PD)])
    _stage(st, 3, AluOp.MIN, _D.PREV_DELAY_0, _D.PREV_ALU_OUT)   # m
    _stage(st, 4, AluOp.MAX, _D.CURR_ALU_OUT, _D.PREV_ALU_OUT)   # running max
    for k in (5, 6, 7):
        _stage(st, k, AluOp.BYPASS, _D.PREV_ALU_OUT)
    st.trigger = (Trigger.SRC_TENSOR_DONE, Trigger.NONE, Trigger.NONE)
    st.next_uop = (0, 0, 0)
    st.out[OutPath.WR0_LO] = OutSel.ALU_OUT
    st.out_enable[OutPath.WR0_LO] = 1
    return [seed, st]


_REGISTERED = {}


def _register_ops():
    if _REGISTERED:
        return _REGISTERED["ttr"], _REGISTERED["tmr"]
    ver = "v3"
    ttr_spec = Spec(body=scan(AluOp.ADD, Src0 * Src1), reference=_ttr_ref)
    tmr_body = minn(Src0, (C0 - scan(AluOp.ADD, C2, init=Zero - C2)) + C1)
    tmr_spec = Spec(body=tmr_body, reference=_tmr_ref)

    ops = []
    for name, spec, uops1x, uops2x, rd1 in (
        ("COSTVOL_TTR2X", ttr_spec, lower(ttr_spec, ver="v3"), _ttr2x_uops(), True),
        ("COSTVOL_TMR2X", tmr_spec, _tmr1x_uops(), _tmr2x_uops(), True),
    ):
        row = 1 + len(dve_ops.OPS)
        assert row < 0x20
        op = DveOp(name, spec, subdim=False, uops_sha={})
        dve_ops.OPS.append(op)
        dve_ops.CUSTOM_DVE_SPECS[name] = spec
        dve_ops._SUB_OPCODE_FOR_NAME[name] = row
        spec_obj = DveOpSpec(
            name=name,
            opcode=row,
            uops=uops1x,
            uops_2x=uops2x if USE_2X else None,
            rd1_en=rd1,
            perf_max=1 if USE_2X else 0,
        )
        spec_obj.validate(ver)
        dve_ops._COMPILE_CACHE[(name, ver)] = spec_obj
        ops.append(op)
    _REGISTERED["ttr"], _REGISTERED["tmr"] = ops
    return ops[0], ops[1]


# --------------------------------------------------------------------------- #
# Kernel body
# --------------------------------------------------------------------------- #


@with_exitstack
def _body(ctx: ExitStack, tc: "tile.TileContext", io: dict):
    nc = tc.nc
    TTR2X, TMR2X = _register_ops()
    lfeat, rfeat = io["lfeat"], io["rfeat"]
    widx, bnd, ivec, scales = io["widx"], io["bnd"], io["ivec"], io["scales"]
    out_dc, out_cf = io["out_dc"], io["out_cf"]

    singles = ctx.enter_context(tc.tile_pool(name="singles", bufs=1))
    feats = ctx.enter_context(tc.tile_pool(name="feats", bufs=6))
    psum = ctx.enter_context(tc.tile_pool(name="psum", bufs=2, space="PSUM"))
    epool = ctx.enter_context(tc.tile_pool(name="epool", bufs=3))
    scr = ctx.enter_context(tc.tile_pool(name="scr", bufs=4))
    acc = ctx.enter_context(tc.tile_pool(name="acc", bufs=6))
    fin = ctx.enter_context(tc.tile_pool(name="fin", bufs=2))

    widx_sb = singles.tile([128, 4, W], F16)
    nc.sync.dma_start(out=widx_sb[:], in_=widx[:, :, :])
    bnd_sb = singles.tile([128, 8], F32)
    nc.sync.dma_start(out=bnd_sb[:], in_=bnd[:, :])
    ivec_sb = singles.tile([128, NT], F32)
    nc.sync.dma_start(out=ivec_sb[:], in_=ivec[:, :])
    sc_sb = singles.tile([128, B], F32)
    nc.gpsimd.dma_start(out=sc_sb[:], in_=scales[:, :].to_broadcast((128, B)))

    def cdve(op, **kw):
        r = nc.vector._custom_dve(op, **kw)
        if USE_2X:
            r.ins.perf_max = 1
        return r

    for b in range(B):
        s1c = acc.tile([128, NT], F32, tag="s1c")
        c1c = acc.tile([128, NT], F32, tag="c1c")
        dnc = acc.tile([128, NT], F32, tag="dnc")
        for h in range(HLOC):
            lf = feats.tile([128, W], F16, tag="lf")
            rf = feats.tile([128, W], F16, tag="rf")
            nc.sync.dma_start(out=lf[:], in_=lfeat[b, :, h, :])
            nc.sync.dma_start(out=rf[:], in_=rfeat[b, :, h, :])
            vol = psum.tile([128, 4 * W], F32, tag="vol")
            e = epool.tile([128, 4 * W], F16, tag="e")
            for mi in range(4):
                nc.tensor.matmul(
                    vol[:, mi * W:(mi + 1) * W],
                    lf[:, mi * 128:(mi + 1) * 128],
                    rf[:, :],
                    start=True,
                    stop=True,
                )
            for mi in range(4):
                t = h * 4 + mi
                nc.scalar.activation(
                    out=e[:, mi * W:(mi + 1) * W],
                    in_=vol[:, mi * W:(mi + 1) * W],
                    func=mybir.ActivationFunctionType.Exp,
                    scale=SCALE,
                    accum_out=dnc[:, t:t + 1],
                )
            # running-reduction outputs; the total lands at column W-1 of
            # each mi page (out slices are right-aligned within the page)
            sow = scr.tile([128, 4, W], F16, tag="sow")
            mow = scr.tile([128, 4, W], F16, tag="mow")
            for mi in range(4):
                ext = (mi + 1) * 128
                off = mi * W
                cdve(
                    TTR2X,
                    out=sow[:, mi, W - ext:],
                    in0=e[:, off:off + ext],
                    in1=widx_sb[:, mi, :ext],
                )
                cdve(
                    TMR2X,
                    out=mow[:, mi, W - ext:],
                    in0=e[:, off:off + ext],
                    in1=e[:, off:off + ext],
                    s0=bnd_sb[:, mi:mi + 1],
                    s1=bnd_sb[:, 4 + mi:5 + mi],
                    imm2=HUGE,
                )
            t0 = h * 4
            nc.vector.tensor_copy(out=s1c[:, t0:t0 + 4], in_=sow[:, :, W - 1:W])
            nc.vector.tensor_copy(out=c1c[:, t0:t0 + 4], in_=mow[:, :, W - 1:W])
        # ---- batched finals for this b ----
        r = fin.tile([128, NT], F32, tag="r")
        nc.vector.reciprocal_approx_fast(out=r[:], in_=dnc[:])
        cf = fin.tile([128, NT], F32, tag="cf")
        nc.gpsimd.tensor_mul(out=cf[:], in0=c1c[:], in1=r[:])
        cor = fin.tile([128, NT], F32, tag="cor")
        nc.gpsimd.tensor_mul(out=cor[:], in0=s1c[:], in1=r[:])
        cor64 = fin.tile([128, NT], F32, tag="cor64")
        nc.gpsimd.tensor_scalar(
            out=cor64[:], in0=cor[:], scalar1=float(WSCALE), scalar2=None,
            op0=mybir.AluOpType.mult,
        )
        dd = fin.tile([128, NT], F32, tag="dd")
        nc.gpsimd.tensor_sub(out=dd[:], in0=ivec_sb[:], in1=cor64[:])
        dcl = fin.tile([128, NT], F32, tag="dcl")
        nc.gpsimd.tensor_scalar(
            out=dcl[:], in0=dd[:], scalar1=1.0 / W, scalar2=MIN_DISP,
            op0=mybir.AluOpType.mult, op1=mybir.AluOpType.max,
        )
        r2 = fin.tile([128, NT], F32, tag="r2")
        nc.vector.reciprocal_approx_fast(out=r2[:], in_=dcl[:])
        od = fin.tile([128, NT], F32, tag="od")
        nc.vector.tensor_scalar(
            out=od[:], in0=r2[:], scalar1=sc_sb[:, b:b + 1], scalar2=None,
            op0=mybir.AluOpType.mult,
        )
        nc.sync.dma_start(out=out_dc[b, :, :], in_=od[:])
        nc.sync.dma_start(out=out_cf[b, :, :], in_=cf[:])


_NC_CACHE = None


def _build_nc():
    global _NC_CACHE
    if _NC_CACHE is not None:
        return _NC_CACHE
    _register_ops()
    nc = bacc.Bacc("TRN2", target_bir_lowering=False, debug=False)
    io = {
        "lfeat": nc.dram_tensor("lfeat", (B, C, HLOC, W), F16, kind="ExternalInput"),
        "rfeat": nc.dram_tensor("rfeat", (B, C, HLOC, W), F16, kind="ExternalInput"),
        "widx": nc.dram_tensor("widx", (128, 4, W), F16, kind="ExternalInput"),
        "bnd": nc.dram_tensor("bnd", (128, 8), F32, kind="ExternalInput"),
        "ivec": nc.dram_tensor("ivec", (128, NT), F32, kind="ExternalInput"),
        "scales": nc.dram_tensor("scales", (1, B), F32, kind="ExternalInput"),
        "out_dc": nc.dram_tensor("out_dc", (B, 128, NT), F32, kind="ExternalOutput"),
        "out_cf": nc.dram_tensor("out_cf", (B, 128, NT), F32, kind="ExternalOutput"),
    }
    with tile.TileContext(nc) as tc:
        _body(tc, io)
    nc.compile()
    _NC_CACHE = nc
    return nc


def _host_constants():
    p = np.arange(128)[:, None, None]
    mi = np.arange(4)[None, :, None]
    j = np.arange(W)[None, None, :]
    widx = np.where(j <= mi * 128 + p, j / WSCALE, 0).astype(np.float16)  # (128,4,W)
    end = (np.arange(4)[None, :] * 128 + np.arange(128)[:, None] + 1)  # (128,4)
    endlo = (end + 1) // 2
    endhi = end // 2
    bnd = np.concatenate([endlo, endhi], axis=1).astype(np.float32) * np.float32(HUGE)
    iv = (np.arange(4)[None, :] * 128 + np.arange(128)[:, None]).astype(np.float32)
    ivec = np.tile(iv, (1, HLOC))                                      # (128, 64)
    return widx, bnd, ivec


def kernel(feat, extri, intri, near, far, _run_kwargs=None, _core_ids=None):
    feat = np.asarray(feat, dtype=np.float32)
    extri = np.asarray(extri, dtype=np.float32)
    intri = np.asarray(intri, dtype=np.float32)
    far = np.asarray(far, dtype=np.float32)

    fx = intri[:, 0, 0, 0]                                             # (B,)
    baseline = np.linalg.norm(extri[:, 0, :3, 3] - extri[:, 1, :3, 3], axis=-1)
    lfar = far[:, 0]
    scales = (fx * baseline / lfar).astype(np.float32).reshape(1, B)

    widx, bnd, ivec = _host_constants()
    core_ids = list(range(NCORES)) if _core_ids is None else _core_ids

    lf16 = feat[:, 0].astype(np.float16)                               # (B,C,H,W)
    rf16 = feat[:, 1].astype(np.float16)

    in_maps = []
    for ci in range(len(core_ids)):
        hs = slice(ci * HLOC, (ci + 1) * HLOC)
        in_maps.append({
            "lfeat": np.ascontiguousarray(lf16[:, :, hs, :]),
            "rfeat": np.ascontiguousarray(rf16[:, :, hs, :]),
            "widx": widx, "bnd": bnd, "ivec": ivec, "scales": scales,
        })

    nc = _build_nc()
    res = run_bass_kernel_spmd(nc, in_maps, core_ids=core_ids,
                               **(_run_kwargs or {}))

    out = np.zeros((B, 1, 2, H, W), dtype=np.float32)
    for ci in range(len(core_ids)):
        h0 = ci * HLOC
        dc = res.results[ci]["out_dc"]          # (B, 128, 64), col = h*4+mi
        cf = res.results[ci]["out_cf"]
        dc = dc.reshape(B, 128, HLOC, 4).transpose(0, 2, 3, 1).reshape(B, HLOC, W)
        cf = cf.reshape(B, 128, HLOC, 4).transpose(0, 2, 3, 1).reshape(B, HLOC, W)
        out[:, 0, 0, h0:h0 + HLOC, :] = dc
        out[:, 0, 1, h0:h0 + HLOC, :] = cf
    if _run_kwargs:
        kernel.last_results = res
    return out
